# revision 41
# baseline (speedup 1.0000x reference)
"""Trainium2 Bass kernel for nn_Decoder_31370441129997.

GRU decoder: 12 sequential steps of (Linear+ReLU) -> 3x GRU cell -> Linear(2),
with the input-layer representation fed back from the last GRU layer's hidden.

Strategy: data-parallel over batch (4096 -> 8 cores x 512). All weights
resident in SBUF as float32r (TF32-like: fp32 with 11-bit mantissa, full PE
rate at moving-free-dim 512). Activations kept feature-major [H, B] so the
recurrence needs no transposes. Gate math in fp32 from PSUM with fused
bias/activation ops (ACT sigmoid/tanh with per-partition bias APs, DVE
scalar_tensor_tensor). Step 0 skips all h-side matmuls (h==0).
"""
import os
import sys

sys.path.insert(0, "/opt/trn_rl_repo")

from contextlib import ExitStack

import numpy as np

import concourse.bass as bass
import concourse.tile as tile
from concourse import bacc, mybir
from concourse.bass_utils import run_bass_kernel_spmd

TPRED = 12
H = 512
L = 3
B = 4096
NCORES = 8
BL = B // NCORES  # 512 batch rows per core
KT = H // 128     # contraction chunks
MT = H // 128     # feature tiles per gate

F32 = mybir.dt.float32
F32R = mybir.dt.float32r
BF16 = mybir.dt.bfloat16
AF = mybir.ActivationFunctionType
ALU = mybir.AluOpType

# compute dtype for matmul operands: f32r (TF32-like, ~4e-4 end-to-end error)
# or bf16 (~5e-3 error, slightly faster weight loads)
MODE = os.environ.get("KERNEL_DTYPE", "f32r")
CDT = F32R if MODE == "f32r" else BF16

_CACHE = {}


def _round_f32r(x: np.ndarray) -> np.ndarray:
    """Round fp32 to the PE's float32r grid (RNE, drop 12 low mantissa bits)."""
    u = np.ascontiguousarray(x, dtype=np.float32).view(np.uint32).astype(np.uint64)
    lsb = (u >> np.uint64(12)) & np.uint64(1)
    u = (u + np.uint64(0x7FF) + lsb) & np.uint64(0xFFFFF000)
    return u.astype(np.uint32).view(np.float32).reshape(x.shape)


def _build(dbg=False):
    assert not dbg or CDT is F32R
    """Build + compile the per-core Bass program (identical on all 8 cores)."""
    nc = bacc.Bacc("TRN2", target_bir_lowering=False, debug=False,
                   dynamic_dma_scratch_size=512)

    rep_d = nc.dram_tensor("rep", [H, BL], CDT, kind="ExternalInput").ap()
    win_d = nc.dram_tensor("win", [H, H], CDT, kind="ExternalInput").ap()
    wx_d = nc.dram_tensor("wx", [L, H, 3 * H], CDT, kind="ExternalInput").ap()
    wh_d = nc.dram_tensor("wh", [L, H, 3 * H], CDT, kind="ExternalInput").ap()
    wout_d = nc.dram_tensor("wout", [H, 2], CDT, kind="ExternalInput").ap()
    bias_d = nc.dram_tensor("bias", [128, 53], F32, kind="ExternalInput").ap()
    out_d = nc.dram_tensor("out", [TPRED, 2, BL], F32, kind="ExternalOutput").ap()
    if dbg:
        dbg_h0_d = nc.dram_tensor("dbg_h0", [L, H, BL], F32,
                                  kind="ExternalOutput").ap()
        dbg_x1_d = nc.dram_tensor("dbg_x1", [H, BL], F32,
                                  kind="ExternalOutput").ap()
        dbg_g_d = nc.dram_tensor("dbg_g", [3, H, BL], F32,
                                 kind="ExternalOutput").ap()
        dbg_h1_d = nc.dram_tensor("dbg_h1", [L, H, BL], F32,
                                  kind="ExternalOutput").ap()

    with tile.TileContext(nc) as tc, ExitStack() as ctx:
        wpool = ctx.enter_context(tc.tile_pool(name="w", bufs=1))
        state = ctx.enter_context(tc.tile_pool(name="state", bufs=1))
        gates = ctx.enter_context(tc.tile_pool(name="gates", bufs=2))
        psum = ctx.enter_context(tc.tile_pool(name="psum", bufs=2, space="PSUM"))

        # DMA order matters: step 0 needs rep + W_in + W_ih (no h-side, no
        # W_out until step 1), so those go first and compute starts ~50us
        # before the W_hh stream finishes.
        h = [[state.tile([128, BL], CDT, tag=f"h{l}_{m}", name=f"h{l}_{m}")
              for m in range(MT)] for l in range(L)]
        for m in range(MT):
            for p in range(0, 128, 64):
                nc.sync.dma_start(h[2][m][p:p + 64, :],
                                  rep_d[m * 128 + p:m * 128 + p + 64, :])
        win = wpool.tile([128, KT, H], CDT, tag="win")
        for k in range(KT):
            for p in range(0, 128, 64):
                nc.sync.dma_start(win[p:p + 64, k, :],
                                  win_d[k * 128 + p:k * 128 + p + 64, :])
        bias = wpool.tile([128, 53], F32, tag="bias")
        nc.sync.dma_start(bias[:], bias_d[:])
        wx = []
        for l in range(L):
            t = wpool.tile([128, KT, 3 * H], CDT, tag=f"wx{l}")
            for k in range(KT):
                nc.sync.dma_start(t[:, k, :], wx_d[l, k * 128:(k + 1) * 128, :])
            wx.append(t)
        wh = []
        for l in range(L):
            t = wpool.tile([128, KT, 3 * H], CDT, tag=f"wh{l}")
            for k in range(KT):
                nc.sync.dma_start(t[:, k, :], wh_d[l, k * 128:(k + 1) * 128, :])
            wh.append(t)
        wout = wpool.tile([128, KT, 2], CDT, tag="wout")
        nc.sync.dma_start(wout[:], wout_d.rearrange("(kt p) c -> p kt c", p=128))

        # hidden state, one [128, BL] tile per (layer, feature-chunk).
        # Within a step, layer gates write h(t) into `ht` temp tiles; GpSimd
        # copies them back into h after every reader of h(t-1) is done
        # (in-place updates would corrupt later feature-chunks' h-side
        # matmuls, which read all chunks of h(t-1)).
        # h[2] doubles as the step-0 representation input.
        x = [state.tile([128, BL], CDT, tag=f"x{m}", name=f"x{m}")
             for m in range(MT)]

        def bcol(c):
            return bias[:, c:c + 1]

        # tiles holding the freshest layer-2 hidden (the gate-written temps,
        # available a GpSimd-copy earlier than h[2] itself)
        h2hot = h[2]

        def outproj(t):
            # b_out is added host-side after the gather
            po = psum.tile([2, BL], F32, tag="z")
            for k in range(KT):
                nc.tensor.matmul(po[:], lhsT=wout[:, k, :], rhs=h2hot[k][:],
                                 start=(k == 0), stop=(k == KT - 1))
            o = gates.tile([2, BL], F32, tag="o", bufs=1)
            nc.scalar.copy(o[:], po[:])
            nc.sync.dma_start(out_d[t], o[:])

        def hside(t, l, m):
            """h-side matmul groups for (layer l, chunk m): all depend only on
            h(t-1), so they are the fill-in work for gate-latency stalls."""
            if t == 0:
                return None, None, None
            hp, whl = h[l], wh[l]
            lo, hi = m * 128, (m + 1) * 128
            ph = psum.tile([128, BL], F32, tag="hn", bufs=3,
                           name=f"ph_{t}_{l}_{m}")
            for k in range(KT):
                nc.tensor.matmul(ph[:], lhsT=whl[:, k, 2 * H + lo:2 * H + hi],
                                 rhs=hp[k][:], start=(k == 0), stop=(k == KT - 1))
            pr = psum.tile([128, BL], F32, tag="r", name=f"pr_{t}_{l}_{m}")
            for k in range(KT):
                nc.tensor.matmul(pr[:], lhsT=whl[:, k, lo:hi],
                                 rhs=hp[k][:], start=(k == 0), stop=False)
            pz = psum.tile([128, BL], F32, tag="z", name=f"pz_{t}_{l}_{m}")
            for k in range(KT):
                nc.tensor.matmul(pz[:], lhsT=whl[:, k, H + lo:H + hi],
                                 rhs=hp[k][:], start=(k == 0), stop=False)
            return ph, pr, pz

        for t in range(TPRED):
            if dbg and t in (1, 2):
                dst = dbg_h0_d if t == 1 else dbg_h1_d
                for l in range(L):
                    for m in range(MT):
                        nc.sync.dma_start(
                            dst[l, m * 128:(m + 1) * 128, :],
                            h[l][m][:].bitcast(F32))
            # L0 chunk-0 h-side first: it only needs h(t-1) and keeps the PE
            # busy while the previous step's layer-2 gate chain finishes
            pend = hside(t, 0, 0)
            if t > 0:
                outproj(t - 1)
            # input layer: x = relu(W_in @ h3 + b_in)
            for m in range(MT):
                px = psum.tile([128, BL], F32, tag="in", bufs=1)
                for k in range(KT):
                    nc.tensor.matmul(px[:],
                                     lhsT=win[:, k, m * 128:(m + 1) * 128],
                                     rhs=h2hot[k][:],
                                     start=(k == 0), stop=(k == KT - 1))
                nc.vector.tensor_scalar(x[m][:], px[:], bcol(48 + m), 0.0,
                                        op0=ALU.add, op1=ALU.max)
                if dbg and t == 1:
                    nc.sync.dma_start(dbg_x1_d[m * 128:(m + 1) * 128, :],
                                      x[m][:].bitcast(F32))
            for l in range(L):
                xin = x if l == 0 else prev_ht
                hp = h[l]
                wxl, whl = wx[l], wh[l]
                ht = []
                for m in range(MT):
                    lo = m * 128
                    hi = lo + 128
                    if m == 0:
                        ph, pr, pz = pend
                    else:
                        ph, pr, pz = hside(t, l, m)
                    if t == 0:
                        pr = psum.tile([128, BL], F32, tag="r",
                                       name=f"pr_{t}_{l}_{m}")
                        pz = psum.tile([128, BL], F32, tag="z",
                                       name=f"pz_{t}_{l}_{m}")
                    for k in range(KT):
                        nc.tensor.matmul(pr[:], lhsT=wxl[:, k, lo:hi],
                                         rhs=xin[k][:],
                                         start=(t == 0 and k == 0),
                                         stop=(k == KT - 1))
                    for k in range(KT):
                        nc.tensor.matmul(pz[:], lhsT=wxl[:, k, H + lo:H + hi],
                                         rhs=xin[k][:],
                                         start=(t == 0 and k == 0),
                                         stop=(k == KT - 1))
                    pin = psum.tile([128, BL], F32, tag="in", bufs=1)
                    for k in range(KT):
                        nc.tensor.matmul(pin[:], lhsT=wxl[:, k, 2 * H + lo:2 * H + hi],
                                         rhs=xin[k][:], start=(k == 0),
                                         stop=(k == KT - 1))
                    if m == MT - 1 and l < L - 1:
                        # next layer's chunk-0 h-side: fills the PE while this
                        # layer's last gate chain produces the next x input
                        pend = hside(t, l + 1, 0)

                    # gates for this feature chunk.
                    # h' = (1-z)*n + z*h computed as e1 + q*n with e1 = z*h
                    # and q = 1-z hoisted off the post-tanh critical chain.
                    r = gates.tile([128, BL], F32, tag="r")
                    nc.scalar.activation(r[:], pr[:], AF.Sigmoid,
                                         bias=bcol(l * 16 + m))
                    z = gates.tile([128, BL], F32, tag="z")
                    nc.scalar.activation(z[:], pz[:], AF.Sigmoid,
                                         bias=bcol(l * 16 + 4 + m))
                    q = gates.tile([128, BL], F32, tag="q", bufs=1)
                    nc.scalar.activation(q[:], z[:], AF.Identity, bias=1.0,
                                         scale=-1.0)
                    if t > 0:
                        e1 = gates.tile([128, BL], F32, tag="e1", bufs=1)
                        nc.vector.tensor_mul(e1[:], z[:],
                                             hp[m][:].bitcast(F32)
                                             if CDT is F32R else hp[m][:])
                    t1 = gates.tile([128, BL], F32, tag="t1")
                    if t > 0:
                        # t1 = (hn_psum + b_hh_n) * r
                        nc.vector.scalar_tensor_tensor(
                            t1[:], ph[:], bcol(l * 16 + 8 + m), r[:],
                            op0=ALU.add, op1=ALU.mult)
                    else:
                        nc.vector.tensor_scalar(t1[:], r[:], bcol(l * 16 + 8 + m),
                                                None, op0=ALU.mult)
                    t2 = gates.tile([128, BL], F32, tag="t1")
                    nc.vector.tensor_add(t2[:], t1[:], pin[:])
                    n = gates.tile([128, BL], F32, tag="n", bufs=1)
                    nc.scalar.activation(n[:], t2[:], AF.Tanh,
                                         bias=bcol(l * 16 + 12 + m))
                    if dbg and t == 1 and l == 0:
                        sl = slice(m * 128, (m + 1) * 128)
                        nc.sync.dma_start(dbg_g_d[0, sl, :], r[:])
                        nc.sync.dma_start(dbg_g_d[1, sl, :], z[:])
                        nc.sync.dma_start(dbg_g_d[2, sl, :], n[:])
                    hnew = gates.tile([128, BL], CDT, tag="ht", bufs=8,
                                      name=f"ht_{t}_{l}_{m}")
                    if t > 0:
                        e2 = gates.tile([128, BL], F32, tag="e2", bufs=1)
                        nc.vector.tensor_mul(e2[:], q[:], n[:])
                        nc.vector.tensor_add(hnew[:], e1[:], e2[:])
                    else:
                        # h' = (1 - z) * n
                        nc.vector.tensor_mul(hnew[:], q[:], n[:])
                    ht.append(hnew)
                # install h(t) only after the whole layer is done: the h-side
                # matmuls of every feature chunk read all chunks of h(t-1),
                # so chunk m must not be overwritten inside the m-loop
                for m in range(MT):
                    nc.gpsimd.tensor_copy(h[l][m][:], ht[m][:])
                prev_ht = ht
                if l == 2:
                    h2hot = ht
        outproj(TPRED - 1)

    nc.compile()
    return nc


def _to_dev(x):
    if CDT is F32R:
        return _round_f32r(x)
    import ml_dtypes
    return np.ascontiguousarray(x).astype(ml_dtypes.bfloat16)


def _prep_inputs(representation, W_in, b_in, W_ih, W_hh, b_ih, b_hh, W_out, b_out):
    rep_T = np.ascontiguousarray(representation.reshape(B, H).T)  # [H, B]
    win = _to_dev(np.ascontiguousarray(W_in.T))                   # [H, H]
    wx = _to_dev(np.ascontiguousarray(np.transpose(W_ih, (0, 2, 1))))
    wh = _to_dev(np.ascontiguousarray(np.transpose(W_hh, (0, 2, 1))))
    wout = _to_dev(np.ascontiguousarray(W_out.T))                 # [H, 2]

    bias = np.zeros((128, 53), dtype=np.float32)
    brz = (b_ih[:, :2 * H] + b_hh[:, :2 * H]).astype(np.float32)  # [L, 2H]
    for l in range(L):
        for g in range(2):
            for m in range(MT):
                bias[:, l * 16 + g * 4 + m] = brz[l, g * H + m * 128:
                                                  g * H + (m + 1) * 128]
        for m in range(MT):
            bias[:, l * 16 + 8 + m] = b_hh[l, 2 * H + m * 128:2 * H + (m + 1) * 128]
            bias[:, l * 16 + 12 + m] = b_ih[l, 2 * H + m * 128:2 * H + (m + 1) * 128]
    for m in range(MT):
        bias[:, 48 + m] = b_in[m * 128:(m + 1) * 128]
    bias[0:2, 52] = b_out

    shared = {"win": win, "wx": wx, "wh": wh, "wout": wout, "bias": bias}
    in_maps = []
    for c in range(NCORES):
        m = dict(shared)
        m["rep"] = _to_dev(np.ascontiguousarray(rep_T[:, c * BL:(c + 1) * BL]))
        in_maps.append(m)
    return in_maps


def _run(inputs, trace=False):
    if "nc" not in _CACHE:
        _CACHE["nc"] = _build()
    nc = _CACHE["nc"]
    in_maps = _prep_inputs(
        inputs["representation"], inputs["W_in"], inputs["b_in"],
        inputs["W_ih"], inputs["W_hh"], inputs["b_ih"], inputs["b_hh"],
        inputs["W_out"], inputs["b_out"])
    res = run_bass_kernel_spmd(nc, in_maps, core_ids=list(range(NCORES)),
                               trace=trace)
    # per-core out: [TPRED, 2, BL] -> full [B, TPRED, 2]
    full = np.empty((B, TPRED, 2), dtype=np.float32)
    for c in range(NCORES):
        o = res.results[c]["out"]                      # [12, 2, BL]
        full[c * BL:(c + 1) * BL] = np.transpose(o, (2, 0, 1))
    full += inputs["b_out"].astype(np.float32)[None, None, :]
    return full, res


def kernel(**inputs) -> np.ndarray:
    out, _ = _run(inputs, trace=False)
    return out


def _setup_tracing():
    """Register the NTFF profile hook shim (test harness only)."""
    import types

    import trn_agent_boot.trn_boot as tb

    mod = types.ModuleType("antenv.axon_hooks")
    hook = [tb._ntff_profile_via_ctypes("/opt/axon/libaxon_pjrt.so")]
    mod.get_axon_ntff_profile_hook = lambda: hook[0]
    mod.set_axon_ntff_profile_hook = lambda h: hook.__setitem__(0, h)
    sys.modules["antenv.axon_hooks"] = mod
    import antenv
    antenv.axon_hooks = mod

    from concourse import bass_utils
    bass_utils.upload_artifacts = lambda tmpdir: str(tmpdir)
